# revision 21
# baseline (speedup 1.0000x reference)
"""Trainium2 Bass kernel for nn_DecoderCell (GRU-style decoder cell).

Reference computation (per batch row):
    r      = sigmoid(x @ Wr.T + hprev @ Ur.T + c @ Cr.T + br)
    z      = sigmoid(x @ Wz.T + hprev @ Uz.T + c @ Cz.T + bz)
    h_prop = tanh  (x @ Wh.T + (r * hprev) @ Uh.T + c @ Ch.T + bh)
    out    = z * h_prop + (1 - z) * hprev

Shapes: B=8192, IN=1024, H=1024, c is [B, 2H].

Strategy:
  - Data-parallel across 8 NeuronCores: batch shard of 1024 rows per core,
    weights replicated. No collectives.
  - All compute in the "transposed domain": per core we hold
    AT = [x | hprev | c].T  -> [4096, 1024]  (k-major: the contraction dim
    lives on SBUF partitions); gate pre-activations come out as [H, B_s]
    tiles, so the r*hprev product needed by the Uh matmul is produced
    directly in k-major layout and no on-device transposes are required.
  - Mixed precision per matmul term: each of the 9 terms (3 gates x
    {W:x, U:hprev, C:c}) runs either as fp16 matmuls (k-tiles of 128) or
    as fp8e4 (e4m3) matmuls in DoubleRow perf mode, which contracts
    k-pairs of 256 per pass at the same per-instruction cost -> 2x PE
    rate for those terms (measured on HW: ~216-220ns/instr both modes).
    The FP8 term set (all of r, z's C term, 6/8 k-tiles of z's W term,
    h's U term) puts the end-to-end RMS relative error at 1.9710e-2,
    under the 2e-2 gate (device matches the numpy quantization sim to
    ~4e-8; per-pair marginal variances from the sim: r terms ~1.5e-6,
    zC 1.8e-5, hU 2.0e-5, zW/zU 2.7e-5, hC 3.8e-5, hW 5.7e-5 -- the
    chosen set is the greedy frontier, and one more pair would land at
    ~2.05e-2, over the gate): fp8 noise flows through saturating
    sigmoid/tanh nonlinearities, and the r-gate is doubly damped (its
    error passes through sigmoid' and then another matmul + tanh).
    HW exec ~243.8us vs 359us for the all-fp16 baseline (PE busy
    ~227us; note the part clock varies run-to-run -- occasional
    slow-state runs measure a few us more for the identical NEFF).
    NOTE the measured rel err depends on the input *realization*:
    jax.random under the axon platform with default_device(cpu) (what
    test.py and the harness use) yields 1.9710e-2; the pure-CPU jax
    realization yields 1.816e-2 for the same kernel.
  - DMA layouts are packed on host so every transfer is per-partition
    contiguous (2KB descriptors; cut packet count 62k -> 19k and lifted
    the early-stream rate to ~390GB/s).  r's epilogue only stores
    sigmoid(pre_r) as fp16; the r*hprev fp8 product is deferred to the
    z groups so r's DMA-limited warm-up window never streams h16.
  - r's first two column-quarter groups are emitted interleaved
    (W/U units alternate between them, C runs jq0 -> epilogue -> jq1):
    both share the same a8 moving tiles, so the DMA demand of the
    warm-up window drops from ~350GB/s (one group alone) to ~210GB/s,
    under the delivery ramp, at no PSUM-turnaround cost.
  - Kernel head: six dependency-free dummy matmuls on a zeroed tile
    run from engine-init (~7us) until the first real operands land
    (~10us) -- the PE p-state clock ramp (~3us to full speed) completes
    on dummy work, so real matmuls start at the full 216ns issue rate
    instead of ramping at ~380-430ns.  The first a8 moving tile is
    DMA'd as two halves and r/W pair-0 is emitted b-major; the first
    weight slabs ride the scalar- and gpsimd-engine DMA queues so they
    aren't queued behind the 256KB moving tile on the sync queue.
  - Kernel tail: exec time tracks (last useful work + fixed ~9us
    teardown - ~6us), so the tail is shaped to finish the final output
    DMA as early as possible: the h/jq3 group's jl1 half is tiled
    512+256 (paired under shared weight loads, so LDWEIGHTS stay
    hidden) plus a lone trailing 256-wide tile; the last tile's
    epilogue runs tanh/mul/add in two 128-col f16 chunks (all-16-bit
    tensor ops run the DVE at 2x) into one [128,256] output tile with
    a single DMA (512B row-packets; a 128-wide DMA's 256B packets
    transfer ~2x slower per byte).  Epilogue hp/zp are f16 everywhere
    (adds ~2e-4 rel on h_prop, negligible through the z gate).
  - All weights are prescaled by 16 on host (fp8's min normal is 2^-6;
    Xavier weights have std 0.031, so unscaled e4m3 would put ~40% of
    the mass in subnormals). The activation instruction computes
    func(in*scale + bias), so the descale by 1/16 is fused and free.
  - PE loop per gate: 4 column-quarter groups of 4 PSUM banks each
    (2 j-tiles x 2 batch-slices of 512), accumulating over the gate's
    k-chunks.  The 8-bank PSUM pool double-buffers groups; weight slabs
    stream from HBM through a prefetch pool.
  - A post-schedule BIR pass removes back-to-back identical fp16
    LDWEIGHTS (the PE keeps stationary weights across matmuls; Tile
    emits one per matmul).  DoubleRow LDWEIGHTS are left alone -- the
    hardware faulted when they were dedup'd.
"""

import sys

sys.path.insert(0, "/opt/trn_rl_repo")

import numpy as np
import ml_dtypes
from contextlib import ExitStack

B = 8192
IN = 1024
H = 1024
NCORES = 8
BS = B // NCORES          # batch rows per core
NB = BS // 512            # 512-wide moving slices per core
KSLAB = 4                 # k-tiles (fp16) or k-pairs (fp8) per weight-slab DMA
WSCALE = 16.0             # host-side weight prescale, descaled in activation

# Which terms run as fp8e4 DoubleRow matmuls. Gate -> subset of "WUC"
# (W: x contribution, U: hprev/r*hprev contribution, C: c contribution).
FP8 = {"r": "WUC", "z": "C", "h": "U"}

# Partial flips: (gate, term) -> number of leading k-tiles (of the term's
# depth) run in fp8, the rest staying fp16.  zW at 6/8 lands the total
# error at 1.9709e-2 on the harness input realization (sim==device to
# ~4e-8; verified against all three jax-platform input realizations,
# worst case 1.9709e-2), keeping a 1.45% margin under the 2e-2 gate
# while shaving ~3.5us more of PE time vs the 4/8 split.
PARTIAL8 = {("z", "W"): 6}

# term -> (k-tile offset in AT, depth in 128-rows)
TERM_K = {"W": (0, 8), "U": (8, 8), "C": (16, 16)}

# Per-gate term issue order.
TERM_ORDER = {"r": "WUC", "z": "WUC", "h": "WUC"}

# Group schedule. r first: it is all-fp8, so its 57us of matmuls only
# need ~10MB of DMA (a8 + fp8 slabs) -- a comfortable warm-up window
# during which the fp16 x16/h16/c8 streams for z arrive. h last (it
# needs every rh tile, i.e. all of r's epilogues).
GROUPS = [
    ("r", 0), ("r", 1), ("r", 2), ("r", 3),
    ("z", 0), ("z", 1), ("z", 2), ("z", 3),
    ("h", 0), ("h", 1), ("h", 2), ("h", 3),
]

_CACHE = {}


def _dedup_ldweights(nc, mybir):
    """Drop redundant fp16 InstLdweights from the scheduled BIR.

    Tile splits every non-fp32 matmul into an explicit LDWEIGHTS + MATMUL
    pair, and the walrus invocation runs with --enable-ldw-opt=false, so
    back-to-back matmuls that reuse the same stationary tile each pay a
    full weight reload. The PE array keeps its weights across matmuls, so
    an LDWEIGHTS identical to the immediately preceding one (and carrying
    no semaphore waits or updates) is a no-op -- remove it.  DoubleRow
    (fp8 perf-mode) LDWEIGHTS are never removed: doing so wedged the PE
    (NRT_EXEC_UNIT_UNRECOVERABLE).
    """
    removed = 0
    for f in nc.m.functions:
        for bb in f.blocks:
            keep = []
            prev_sig = None
            for inst in bb.instructions:
                tn = type(inst).__name__
                if getattr(inst, "engine", None) == mybir.EngineType.PE:
                    if tn == "InstLdweights":
                        if getattr(inst, "perf_mode", None) is not None:
                            prev_sig = None
                            keep.append(inst)
                            continue
                        sig = str(inst.ins[0]) if inst.ins else None
                        si = inst.sync_info
                        clean = si is None or (
                            len(si.on_wait) == 0 and len(si.on_update) == 0
                        )
                        if sig is not None and sig == prev_sig and clean:
                            removed += 1
                            continue
                        prev_sig = sig
                    elif tn in ("InstMatmult", "InstEventSemaphore", "InstNoOp"):
                        pass  # these don't disturb the loaded weights
                    else:
                        prev_sig = None
                keep.append(inst)
            bb.instructions[:] = keep
    return removed


def _build_nc():
    import concourse.bacc as bacc
    import concourse.tile as tile
    from concourse import mybir

    f32 = mybir.dt.float32
    f16 = mybir.dt.float16
    f8 = mybir.dt.float8e4
    DR = mybir.MatmulPerfMode.DoubleRow
    SIG = mybir.ActivationFunctionType.Sigmoid
    TANH = mybir.ActivationFunctionType.Tanh

    nc = bacc.Bacc("TRN2", target_bir_lowering=False, debug=False)

    # Weight tensors are pre-arranged on host into slab-major layouts so
    # every slab DMA is per-partition contiguous (2KB descriptors instead
    # of the 256-512B fragments a strided column slice would produce):
    #   fp8 : [4*128, npair*512]  row = jq*128 + p, col = (pl*2 + i)*256 + j
    #   fp16: [4*128, nkt*256]    row = jq*128 + p, col = kl*256 + j
    # at8 likewise is pair-major: [16*128, 2*BS], row = pl*128 + p.
    at16_d = nc.dram_tensor("at16", [4 * IN, BS], f16, kind="ExternalInput")
    at8_d = nc.dram_tensor("at8", [16 * 128, 2 * BS], f8, kind="ExternalInput")
    w_d = {}
    for g in "rzh":
        for t in "WUC":
            depth = TERM_K[t][1]  # k-tiles
            if (g, t) in PARTIAL8:
                n8 = PARTIAL8[(g, t)]
                w_d[(g, t, 8)] = nc.dram_tensor(
                    f"w{g}{t}8", [4 * 128, (n8 // 2) * 512], f8, kind="ExternalInput"
                )
                w_d[(g, t, 16)] = nc.dram_tensor(
                    f"w{g}{t}16", [4 * 128, (depth - n8) * 256], f16, kind="ExternalInput"
                )
            else:
                if t in FP8[g]:
                    w_d[(g, t)] = nc.dram_tensor(
                        f"w{g}{t}", [4 * 128, (depth // 2) * 512], f8, kind="ExternalInput"
                    )
                else:
                    w_d[(g, t)] = nc.dram_tensor(
                        f"w{g}{t}", [4 * 128, depth * 256], f16, kind="ExternalInput"
                    )
    b_d = {
        g: nc.dram_tensor(f"b{g}", [128, 8], f32, kind="ExternalInput")
        for g in "rzh"
    }
    # fp16 output: halves the tail DMA drain; adds only ~2e-4 RMS rounding
    out_d = nc.dram_tensor("out_t", [H, BS], f16, kind="ExternalOutput")

    hU8 = "U" in FP8["h"]

    with tile.TileContext(nc) as tc:
        with ExitStack() as ctx:
            pp = ctx.enter_context(tc.tile_pool(name="persist", bufs=1))
            wp = ctx.enter_context(tc.tile_pool(name="wslab", bufs=6))
            rp = ctx.enter_context(tc.tile_pool(name="rtmp", bufs=2))
            hpp = ctx.enter_context(tc.tile_pool(name="hprop", bufs=3))
            op = ctx.enter_context(tc.tile_pool(name="otile", bufs=4))
            psp = ctx.enter_context(tc.tile_pool(name="ps", bufs=8, space="PSUM"))

            at16_t = [
                pp.tile([128, BS], f16, tag=f"at{k}", name=f"at{k}")
                for k in range(32)
            ]
            a8_t = [
                pp.tile([128, 2, BS], f8, tag=f"a8_{p}", name=f"a8_{p}")
                for p in range(16)
            ]
            if hU8:
                rh8_t = [
                    pp.tile([128, 2, BS], f8, tag=f"rh{q}", name=f"rh{q}")
                    for q in range(4)
                ]
            else:
                rh16_t = [
                    pp.tile([128, BS], f16, tag=f"rh{j}", name=f"rh{j}")
                    for j in range(8)
                ]
            # r-gate sigmoid outputs; the r*hprev product is deferred to the
            # z groups so r's warm-up window never has to stream h16
            r_t = [
                [pp.tile([128, 512], f16, tag=f"rg{j}_{b}", name=f"rg{j}_{b}") for b in range(NB)]
                for j in range(8)
            ]
            z_t = [
                [pp.tile([128, 512], f16, tag=f"z{j}_{b}", name=f"z{j}_{b}") for b in range(NB)]
                for j in range(8)
            ]
            # w = (1-z)*hprev, precomputed at the z epilogue so the h
            # epilogue (the kernel's tail) is only mul+add
            w_t = [
                [pp.tile([128, 512], f16, tag=f"wz{j}_{b}", name=f"wz{j}_{b}") for b in range(NB)]
                for j in range(8)
            ]
            bias_t = {g: pp.tile([128, 8], f32, tag=f"bias{g}", name=f"bias{g}") for g in "rzh"}

            at16_dma = [None] * 32
            a8_dma = [None] * 16

            def ensure_a16(k):
                if at16_dma[k] is None:
                    at16_dma[k] = nc.sync.dma_start(
                        at16_t[k][:], at16_d.ap()[k * 128:(k + 1) * 128, :]
                    )
                return at16_dma[k]

            def ensure_a8(p):
                if a8_dma[p] is None:
                    src = at8_d.ap()[p * 128:(p + 1) * 128, :].rearrange(
                        "q (two b) -> q two b", two=2
                    )
                    a8_dma[p] = nc.sync.dma_start(a8_t[p][:], src)
                return a8_dma[p]

            def preissue_a8_halves(p):
                # Issue the first moving tile as two 128KB halves so the
                # kernel's first matmuls (batch cols 0:512) wait on half the
                # transfer during the DMA ramp.  (Quarter granularity with
                # 256-wide matmuls measured FASTER but produced corrupted
                # r-gate output on device -- a subtile-dependency race -- so
                # stay at halves, which verify bit-exact.)
                if a8_dma[p] is None:
                    src = at8_d.ap()[p * 128:(p + 1) * 128, :].rearrange(
                        "q (two b) -> q two b", two=2
                    )
                    d0 = nc.sync.dma_start(a8_t[p][:, :, 0:512], src[:, :, 0:512])
                    nc.sync.dma_start(a8_t[p][:, :, 512:BS], src[:, :, 512:BS])
                    a8_dma[p] = d0

            bias_loaded = [False]

            def ensure_bias():
                # biases are first needed at the first epilogue; keep them
                # out of the critical DMA prefix
                if not bias_loaded[0]:
                    for g in "rzh":
                        nc.sync.dma_start(bias_t[g][:], b_d[g].ap()[:, :])
                    bias_loaded[0] = True

            def term_segments(g, t):
                # -> [(is8, w_d key, AT k-tile offset, depth in k-tiles)]
                koff, dep = TERM_K[t]
                if (g, t) in PARTIAL8:
                    n8 = PARTIAL8[(g, t)]
                    return [
                        (True, (g, t, 8), koff, n8),
                        (False, (g, t, 16), koff + n8, dep - n8),
                    ]
                return [(t in FP8[g], (g, t), koff, dep)]

            def fp8_slab_sizes(g, jq, t, npair):
                # The paired r/jq0+jq1 W terms open the kernel while the
                # DMA ramp is still climbing: single-pair slabs there so
                # each matmul only waits on ~64KB of weights
                if g == "r" and jq in (0, 1) and t == "W":
                    sizes = [1] * 4
                    rem = npair - 4
                else:
                    sizes, rem = [], npair
                while rem > 0:
                    s = min(KSLAB, rem)
                    sizes.append(s)
                    rem -= s
                return sizes

            def do_group(g, jq):
                if g == "z":
                    # spread the 4MB c16 stream (needed by h's C term) across
                    # the z groups so h/jq0 doesn't stall on a DMA burst
                    for k in range(16 + 4 * jq, 20 + 4 * jq):
                        ensure_a16(k)
                    # deferred r*hprev products for this jq's j-tiles (the
                    # vector engine is idle here and h16 is resident for zU)
                    for jt in (2 * jq, 2 * jq + 1):
                        ensure_a16(8 + jt)
                        for b in range(NB):
                            bsl = slice(b * 512, (b + 1) * 512)
                            if hU8:
                                dst = rh8_t[jt // 2][:, jt % 2, bsl]
                            else:
                                dst = rh16_t[jt][:, bsl]
                            nc.vector.tensor_mul(
                                dst, r_t[jt][b][:], at16_t[8 + jt][:, bsl]
                            )
                # output tiles: j in [jq*128*2, ...), all BS batch cols
                ps = {}
                for jl in range(2):
                    for b in range(NB):
                        ps[(jl, b)] = psp.tile([128, 512], f32, tag="ps",
                                               name=f"ps_{g}_{jq}_{jl}_{b}")
                # count matmul units (one matmul per (jl, b) each)
                nu = sum(
                    dep // (2 if is8 else 1)
                    for t in "WUC"
                    for is8, _, _, dep in term_segments(g, t)
                )
                ui = 0
                for t in TERM_ORDER[g]:
                    for is8, wkey, koff, dep in term_segments(g, t):
                        if is8:
                            npair = dep // 2
                            poff = koff // 2
                            p0 = 0
                            for ss, sl in enumerate(fp8_slab_sizes(g, jq, t, npair)):
                                slab = wp.tile([128, sl, 2, 256], f8, tag="w8",
                                               name=f"w8_{g}{t}_{jq}_{ss}")
                                src = w_d[wkey].ap()[
                                    jq * 128:(jq + 1) * 128,
                                    p0 * 512:(p0 + sl) * 512,
                                ].rearrange("p (a two j) -> p a two j", a=sl, two=2)
                                nc.sync.dma_start(slab[:], src)
                                for dp in range(sl):
                                    pl = p0 + dp
                                    if g == "h" and t == "U":
                                        mov = rh8_t[pl]
                                    else:
                                        ensure_a8(poff + pl)
                                        mov = a8_t[poff + pl]
                                    for jl in range(2):
                                        lhsT = slab[:, dp, :, jl * 128:(jl + 1) * 128]
                                        for b in range(NB):
                                            nc.tensor.matmul(
                                                ps[(jl, b)][:],
                                                lhsT,
                                                mov[:, :, b * 512:(b + 1) * 512],
                                                start=(ui == 0),
                                                stop=(ui == nu - 1),
                                                perf_mode=DR,
                                            )
                                    ui += 1
                                p0 += sl
                        else:
                            for ks in range((dep + KSLAB - 1) // KSLAB):
                                k0 = ks * KSLAB
                                sl = min(KSLAB, dep - k0)
                                slab = wp.tile([128, sl, 256], f16, tag="w16",
                                               name=f"w16_{g}{t}_{jq}_{ks}")
                                src = w_d[wkey].ap()[
                                    jq * 128:(jq + 1) * 128,
                                    k0 * 256:(k0 + sl) * 256,
                                ].rearrange("p (a j) -> p a j", a=sl)
                                nc.sync.dma_start(slab[:], src)
                                for dk in range(sl):
                                    kl = k0 + dk
                                    if g == "h" and t == "U":
                                        mov = rh16_t[kl]
                                    else:
                                        ensure_a16(koff + kl)
                                        mov = at16_t[koff + kl]
                                    for jl in range(2):
                                        lhsT = slab[:, dk, jl * 128:(jl + 1) * 128]
                                        for b in range(NB):
                                            nc.tensor.matmul(
                                                ps[(jl, b)][:],
                                                lhsT,
                                                mov[:, b * 512:(b + 1) * 512],
                                                start=(ui == 0),
                                                stop=(ui == nu - 1),
                                            )
                                    ui += 1
                assert ui == nu
                ensure_bias()
                group_epilogue(g, jq, ps)

            def group_epilogue(g, jq, ps):
                # ps keyed (jl, b)
                for jl in range(2):
                    jt = 2 * jq + jl
                    for b in range(NB):
                        pst = ps[(jl, b)]
                        bias_ap = bias_t[g][:, jt:jt + 1]
                        bsl = slice(b * 512, (b + 1) * 512)
                        if g == "r":
                            nc.scalar.activation(r_t[jt][b][:], pst[:], SIG,
                                                 bias=bias_ap, scale=1.0 / WSCALE)
                        elif g == "z":
                            ensure_a16(8 + jt)
                            nc.scalar.activation(z_t[jt][b][:], pst[:], SIG,
                                                 bias=bias_ap, scale=1.0 / WSCALE)
                            hT = at16_t[8 + jt][:, bsl]
                            tmp = rp.tile([128, 512], f32, tag="rt", name=f"zh_{jt}_{b}")
                            # w = (1-z)*h = h - z*h
                            nc.vector.tensor_mul(tmp[:], z_t[jt][b][:], hT)
                            nc.vector.tensor_sub(w_t[jt][b][:], hT, tmp[:])
                        else:
                            # f16 hp/zp keep every tensor op all-16-bit (2x
                            # DVE rate); adds only ~2e-4 rel rounding on hp,
                            # negligible through the z gate
                            hp = hpp.tile([128, 512], f16, tag="hp16", name=f"hp_{jt}_{b}")
                            nc.scalar.activation(hp[:], pst[:], TANH,
                                                 bias=bias_ap, scale=1.0 / WSCALE)
                            ot = op.tile([128, 512], f16, tag="ot", name=f"ot_{jt}_{b}")
                            zp = rp.tile([128, 512], f16, tag="zp16", name=f"zp_{jt}_{b}")
                            # out = z*hp + (1-z)*h, with (1-z)*h precomputed
                            nc.vector.tensor_mul(zp[:], z_t[jt][b][:], hp[:])
                            nc.vector.tensor_add(ot[:], zp[:], w_t[jt][b][:])
                            nc.sync.dma_start(
                                out_d.ap()[jt * 128:(jt + 1) * 128, bsl], ot[:]
                            )

            def do_group_pair(g, jqA, jqB):
                # Interleaved emission for two column-quarter groups of an
                # all-fp8 gate (used for r's warm-up): W and U units
                # alternate between the groups, so the first ~14us of PE
                # work only needs the 2MB of x/h a8 pairs plus 1MB of
                # slabs -- always under the DMA ramp -- instead of jqA
                # alone demanding 5MB in 14us.  The C term runs
                # jqA -> epilogue(jqA) -> jqB -> epilogue(jqB) so jqA's
                # PSUM banks free while jqB's C matmuls run.
                assert all(t in FP8[g] for t in "WUC")
                assert not any((g, t) in PARTIAL8 for t in "WUC")
                ps = {}
                for jq in (jqA, jqB):
                    for jl in range(2):
                        for b in range(NB):
                            ps[(jq, jl, b)] = psp.tile(
                                [128, 512], f32, tag="ps",
                                name=f"ps_{g}_{jq}_{jl}_{b}")
                nu = sum(TERM_K[t][1] // 2 for t in "WUC")
                ui = {jqA: 0, jqB: 0}

                def emit(jq, t):
                    koff, dep = TERM_K[t]
                    npair = dep // 2
                    poff = koff // 2
                    p0 = 0
                    for ss, sl in enumerate(fp8_slab_sizes(g, jq, t, npair)):
                        slab = wp.tile([128, sl, 2, 256], f8, tag="w8",
                                       name=f"w8_{g}{t}_{jq}_{ss}")
                        src = w_d[(g, t)].ap()[
                            jq * 128:(jq + 1) * 128,
                            p0 * 512:(p0 + sl) * 512,
                        ].rearrange("p (a two j) -> p a two j", a=sl, two=2)
                        nc.sync.dma_start(slab[:], src)
                        for dp in range(sl):
                            pl = p0 + dp
                            ensure_a8(poff + pl)
                            mov = a8_t[poff + pl]
                            for jl in range(2):
                                lhsT = slab[:, dp, :, jl * 128:(jl + 1) * 128]
                                for b in range(NB):
                                    nc.tensor.matmul(
                                        ps[(jq, jl, b)][:],
                                        lhsT,
                                        mov[:, :, b * 512:(b + 1) * 512],
                                        start=(ui[jq] == 0),
                                        stop=(ui[jq] == nu - 1),
                                        perf_mode=DR,
                                    )
                            ui[jq] += 1
                        p0 += sl

                # --- W warm-up ---
                # The kernel's first matmuls.  Three DMA queues in parallel
                # (sync: a8 halves, scalar: jqA slab, gpsimd: jqB slab) so the
                # first 64KB slab isn't queued behind 256KB of moving data,
                # and pair-0 is emitted b-major so the b=0 matmuls only wait
                # on the first 128KB half of the a8 tile.  Pulls the first
                # matmul from ~11.1us to ~9us into the kernel.
                koff_w, dep_w = TERM_K["W"]
                npair_w = dep_w // 2
                # PE clock pre-warm: the tensor engine only reaches full
                # clock after ~3us of continuous execution.  Six dependency-
                # free matmuls on a zeroed tile keep the PE busy from
                # engine-init (~7us) until the first real operands land
                # (~10us), so the p-state ramp completes on dummy work and
                # the real matmul stream starts at full clock.  Their
                # garbage PSUM writes are overwritten by pair-0's
                # start=True matmuls.
                warm = pp.tile([128, 512], f16, tag="warm", name="warm")
                nc.vector.memset(warm[:], 0)
                worder = [(jq, jl, b) for b in range(NB)
                          for jq in (jqA, jqB) for jl in range(2)]
                for (jq, jl, b) in worder[:6]:
                    nc.tensor.matmul(ps[(jq, jl, b)][:], warm[:, 0:128],
                                     warm[:], start=True, stop=True)
                preissue_a8_halves(0)
                first_slabs = {}
                for jq, eng in ((jqA, nc.scalar), (jqB, nc.gpsimd)):
                    slab = wp.tile([128, 1, 2, 256], f8, tag="w8",
                                   name=f"w8_{g}W_{jq}_0")
                    src = w_d[(g, "W")].ap()[
                        jq * 128:(jq + 1) * 128, 0:512,
                    ].rearrange("p (a two j) -> p a two j", a=1, two=2)
                    eng.dma_start(slab[:], src)
                    first_slabs[jq] = slab
                # pair 0 emitted b-major: the b=0 matmuls only wait on the
                # first a8 half + the 64KB jqA slab
                for b in range(NB):
                    for jq in (jqA, jqB):
                        for jl in range(2):
                            lhsT = first_slabs[jq][:, 0, :, jl * 128:(jl + 1) * 128]
                            nc.tensor.matmul(
                                ps[(jq, jl, b)][:], lhsT,
                                a8_t[0][:, :, b * 512:(b + 1) * 512],
                                start=True, stop=False, perf_mode=DR,
                            )
                for jq in (jqA, jqB):
                    ui[jq] += 1
                for pl in range(1, npair_w):
                    ensure_a8(pl)
                    for jq in (jqA, jqB):
                        slab = wp.tile([128, 1, 2, 256], f8, tag="w8",
                                       name=f"w8_{g}W_{jq}_{pl}")
                        src = w_d[(g, "W")].ap()[
                            jq * 128:(jq + 1) * 128, pl * 512:(pl + 1) * 512,
                        ].rearrange("p (a two j) -> p a two j", a=1, two=2)
                        nc.sync.dma_start(slab[:], src)
                        for jl in range(2):
                            lhsT = slab[:, 0, :, jl * 128:(jl + 1) * 128]
                            for b in range(NB):
                                nc.tensor.matmul(
                                    ps[(jq, jl, b)][:], lhsT,
                                    a8_t[pl][:, :, b * 512:(b + 1) * 512],
                                    start=False, stop=False, perf_mode=DR,
                                )
                        ui[jq] += 1
                emit(jqA, "U")
                emit(jqB, "U")
                emit(jqA, "C")
                ensure_bias()
                group_epilogue(g, jqA, {(jl, b): ps[(jqA, jl, b)]
                                        for jl in range(2) for b in range(NB)})
                emit(jqB, "C")
                group_epilogue(g, jqB, {(jl, b): ps[(jqB, jl, b)]
                                        for jl in range(2) for b in range(NB)})

            def do_group_tail(g, jq):
                # Last group of the kernel: issue matmuls per PSUM tile
                # (jl, b outer, k inner) so three of the four epilogues
                # overlap remaining matmuls instead of serializing at the
                # very end.  All weight slabs are staged up front in
                # persistent tiles.
                units = []  # (is8, slab, idx, mov)
                for t in TERM_ORDER[g]:
                    koff, dep = TERM_K[t]
                    is8 = t in FP8[g]
                    if is8:
                        npair = dep // 2
                        poff = koff // 2
                        slab = pp.tile([128, npair, 2, 256], f8,
                                       tag=f"tw8{t}", name=f"tw8_{g}{t}_{jq}")
                        src = w_d[(g, t)].ap()[
                            jq * 128:(jq + 1) * 128, :
                        ].rearrange("p (a two j) -> p a two j", a=npair, two=2)
                        nc.sync.dma_start(slab[:], src)
                        for pl in range(npair):
                            if g == "h" and t == "U":
                                mov = rh8_t[pl]
                            else:
                                ensure_a8(poff + pl)
                                mov = a8_t[poff + pl]
                            units.append((True, slab, pl, mov))
                    else:
                        nkt = dep
                        slab = pp.tile([128, nkt, 256], f16,
                                       tag=f"tw16{t}", name=f"tw16_{g}{t}_{jq}")
                        src = w_d[(g, t)].ap()[
                            jq * 128:(jq + 1) * 128, :
                        ].rearrange("p (a j) -> p a j", a=nkt)
                        nc.sync.dma_start(slab[:], src)
                        for kl in range(nkt):
                            if g == "h" and t == "U":
                                mov = rh16_t[kl]
                            else:
                                ensure_a16(koff + kl)
                                mov = at16_t[koff + kl]
                            units.append((False, slab, kl, mov))
                ensure_bias()
                nu = len(units)
                # jl1's upper batch half is split into two 256-col PSUM
                # tiles emitted pairwise under shared weight loads, so the
                # kernel's final epilogue + output DMA covers 64KB->32KB
                # and the last HBM transfer finishes ~2us sooner.  (The
                # pairing keeps LDWEIGHTS hidden: each weight load feeds
                # two 256-wide matmuls.)
                for jl in range(2):
                    jt = 2 * jq + jl
                    if jl == 0:
                        tgroups = [[(0, 512)], [(512, 512)]]
                    else:
                        # (512:768) rides with the big tile (its 32KB DMA
                        # then overlaps the last tile's matmuls); the last
                        # tile stands alone so only 64KB->32KB remains
                        # after the final matmul
                        tgroups = [[(0, 512), (512, 256)], [(768, 256)]]
                    for tg in tgroups:
                        pss = {
                            off: psp.tile([128, w], f32, tag="ps",
                                          name=f"ps_{g}_{jq}_{jl}_{off}")
                            for off, w in tg
                        }
                        for ui, (is8, slab, d, mov) in enumerate(units):
                            for off, w in tg:
                                if is8:
                                    nc.tensor.matmul(
                                        pss[off][:],
                                        slab[:, d, :, jl * 128:(jl + 1) * 128],
                                        mov[:, :, off:off + w],
                                        start=(ui == 0), stop=(ui == nu - 1),
                                        perf_mode=DR,
                                    )
                                else:
                                    nc.tensor.matmul(
                                        pss[off][:],
                                        slab[:, d, jl * 128:(jl + 1) * 128],
                                        mov[:, off:off + w],
                                        start=(ui == 0), stop=(ui == nu - 1),
                                    )
                        bias_ap = bias_t[g][:, jt:jt + 1]
                        for off, w in tg:
                            # the very last tile (jl1, 768:1024) pipelines
                            # ACT/TT in two 128-col chunks (f16 ops keep the
                            # DVE in 2x mode) but issues ONE [128,256] DMA:
                            # 512B row-packets transfer ~2x faster per byte
                            # than the 256B packets a 128-wide DMA would use
                            last = (jl == 1 and off == 768)
                            nchunk = 2 if last else 1
                            cw = w // nchunk
                            ot = op.tile([128, w], f16, tag="ot",
                                         name=f"tot_{jt}_{off}")
                            for ci in range(nchunk):
                                o2 = off + ci * cw
                                b2 = o2 // 512
                                i2 = slice(o2 - b2 * 512, o2 - b2 * 512 + cw)
                                hp = hpp.tile([128, cw], f16, tag="hp16",
                                              name=f"thp_{jt}_{o2}")
                                nc.scalar.activation(
                                    hp[:], pss[off][:, ci * cw:(ci + 1) * cw],
                                    TANH, bias=bias_ap, scale=1.0 / WSCALE)
                                zp = rp.tile([128, cw], f16, tag="zp16",
                                             name=f"tzp_{jt}_{o2}")
                                nc.vector.tensor_mul(zp[:], z_t[jt][b2][:, i2], hp[:])
                                nc.vector.tensor_add(ot[:, ci * cw:(ci + 1) * cw],
                                                     zp[:], w_t[jt][b2][:, i2])
                            # sync queue: it spreads packets across all 16
                            # DMA engines (gpsimd's queue measured ~2x
                            # slower on the final transfer)
                            nc.sync.dma_start(
                                out_d.ap()[jt * 128:(jt + 1) * 128,
                                           off:off + w], ot[:]
                            )

            do_group_pair("r", 0, 1)
            for g, jq in GROUPS[2:-1]:
                do_group(g, jq)
            do_group_tail(*GROUPS[-1])

    _dedup_ldweights(nc, mybir)
    nc.finalize()
    return nc


def _get_nc():
    if "nc" not in _CACHE:
        _CACHE["nc"] = _build_nc()
    return _CACHE["nc"]


def _host_prep(inputs):
    x = np.asarray(inputs["x"], dtype=np.float32)
    hprev = np.asarray(inputs["hprev"], dtype=np.float32)
    c = np.asarray(inputs["c"], dtype=np.float32)
    A = np.concatenate([x, hprev, c], axis=1)                  # [B, 4096]
    AF = np.ascontiguousarray(A.T)                             # [4096, B]
    at16 = AF.astype(np.float16)
    at8 = AF.astype(ml_dtypes.float8_e4m3)
    wnames = {
        ("r", "W"): "Wr", ("r", "U"): "Ur", ("r", "C"): "Cr",
        ("z", "W"): "Wz", ("z", "U"): "Uz", ("z", "C"): "Cz",
        ("h", "W"): "Wh", ("h", "U"): "Uh", ("h", "C"): "Ch",
    }
    # slab-major packing: row = jq*128 + p; fp8 col = (pl*2+i)*256 + j,
    # fp16 col = kl*256 + j  (see the dram_tensor comment in _build_nc)
    def pack8(Mk):
        npair = Mk.shape[0] // 256
        A = Mk.reshape(npair, 2, 128, 4, 256).transpose(3, 2, 0, 1, 4)
        return np.ascontiguousarray(
            A.reshape(4 * 128, npair * 512).astype(ml_dtypes.float8_e4m3)
        )

    def pack16(Mk):
        nk = Mk.shape[0] // 128
        A = Mk.reshape(nk, 128, 4, 256).transpose(2, 1, 0, 3)
        return np.ascontiguousarray(
            A.reshape(4 * 128, nk * 256).astype(np.float16)
        )

    w = {}
    for (g, t), nm in wnames.items():
        M = np.asarray(inputs[nm], np.float32).T * WSCALE
        if (g, t) in PARTIAL8:
            s = PARTIAL8[(g, t)] * 128
            w[f"w{g}{t}8"] = pack8(M[:s])
            w[f"w{g}{t}16"] = pack16(M[s:])
        elif t in FP8[g]:
            w[f"w{g}{t}"] = pack8(M)
        else:
            w[f"w{g}{t}"] = pack16(M)
    bias = {
        g: np.ascontiguousarray(
            np.asarray(inputs["b" + g], dtype=np.float32).reshape(8, 128).T
        )
        for g in "rzh"
    }
    return at16, at8, w, bias


def _in_maps(inputs):
    at16, at8, w, bias = _host_prep(inputs)
    maps = []
    for s in range(NCORES):
        a8c = at8[:, s * BS:(s + 1) * BS]
        # pair-major: row = pl*128 + p, col = i*BS + b
        a8c = np.ascontiguousarray(
            a8c.reshape(16, 2, 128, BS).transpose(0, 2, 1, 3).reshape(16 * 128, 2 * BS)
        )
        m = {
            "at16": np.ascontiguousarray(at16[:, s * BS:(s + 1) * BS]),
            "at8": a8c,
            "br": bias["r"],
            "bz": bias["z"],
            "bh": bias["h"],
        }
        m.update(w)
        maps.append(m)
    return maps


def run_device(inputs, trace=False, **kwargs):
    """Run the SPMD kernel; returns (full_output, BassKernelResults)."""
    from concourse.bass_utils import run_bass_kernel_spmd

    nc = _get_nc()
    res = run_bass_kernel_spmd(
        nc, _in_maps(inputs), core_ids=list(range(NCORES)), trace=trace, **kwargs
    )
    out = np.empty((B, H), dtype=np.float32)
    for s in range(NCORES):
        out[s * BS:(s + 1) * BS, :] = res.results[s]["out_t"].T.astype(np.float32)
    return out, res


def kernel(**inputs):
    out, _ = run_device(inputs, trace=False)
    return out

